# revision 1
# baseline (speedup 1.0000x reference)
"""Trainium kernel for nn_AttModel: DCT-window attention + spatio-temporal GCNs.

Strategy: pure data parallel (sharding_hint). Batch 512 is split across the
8 NeuronCores (64 samples each); all conv/GCN/attention weights are
replicated. The per-shard forward runs on each core via PJRT; results are
gathered and concatenated to the full batch.

Self-contained: all shapes/constants hardcoded (B=512, input_n=50,
output_n=25, itera=1, KS=10, DCT_N=10, D=512, F=48).
"""

import numpy as np
import jax
import jax.numpy as jnp

KS = 10
DCT_N = 10
D_MODEL = 512
IN_F = 48
BN_EPS = 1e-5
N_CORES = 8
BATCH = 512
INPUT_N = 50
OUTPUT_N = 25
VL = KS + OUTPUT_N          # 35
VN = INPUT_N - KS - OUTPUT_N + 1  # 16


def _get_dct_matrix(N):
    i = np.arange(N)
    k = np.arange(N)[:, None]
    w = np.full((N, 1), np.sqrt(2.0 / N))
    w[0] = np.sqrt(1.0 / N)
    dct = w * np.cos(np.pi * (i + 0.5) * k / N)
    return dct.astype(np.float32), np.linalg.inv(dct).astype(np.float32)


_DCT_NP, _IDCT_NP = _get_dct_matrix(VL)


def _conv1d(x, w):
    # x: [B, Cin, L], w: [Cout, Cin, K], VALID, stride 1, no bias
    return jax.lax.conv_general_dilated(
        x, w, (1,), 'VALID', dimension_numbers=('NCH', 'OIH', 'NCH'))


def _gconv(g, x):
    # y = att @ (x @ W) + b ; x: [B, node_n, F_in]
    return jnp.einsum('nm,bmf->bnf', g['att'], x @ g['w']) + g['b']


def _bn(y, gamma, beta):
    b, n, f = y.shape
    inv = 1.0 / jnp.sqrt(1.0 + BN_EPS)
    return ((y.reshape(b, -1) * inv) * gamma + beta).reshape(b, n, f)


def _gcn_fwd(p, x):
    y = jnp.tanh(_bn(_gconv(p['gc1'], x), p['bn1_g'], p['bn1_b']))
    for blk in p['blocks']:
        r = y
        z = jnp.tanh(_bn(_gconv(blk['gc1'], y), blk['bn1_g'], blk['bn1_b']))
        z = jnp.tanh(_bn(_gconv(blk['gc2'], z), blk['bn2_g'], blk['bn2_b']))
        y = z + r
    return _gconv(p['gc7'], y) + x


def _forward(src, w):
    """Per-shard forward. src: [b, 50, 48] fp32. Returns (out0, out1, out2, score)."""
    dct_m = jnp.asarray(_DCT_NP)
    idct_m = jnp.asarray(_IDCT_NP)
    dct_n = DCT_N
    bs = src.shape[0]

    src_key = jnp.swapaxes(src, 1, 2)[:, :, :INPUT_N - OUTPUT_N]   # [b,48,25]
    src_query = jnp.swapaxes(src, 1, 2)[:, :, -KS:]                # [b,48,10]
    widx = np.arange(VL)[None, :] + np.arange(VN)[:, None]         # [16,35]
    val = src[:, widx]                                             # [b,16,35,48]
    src_value = jnp.einsum('kt,bvtf->bvfk', dct_m[:dct_n], val).reshape(bs, VN, -1)

    def convQK(x, w1, w2):
        return jax.nn.relu(_conv1d(jax.nn.relu(_conv1d(x, w1)), w2))

    key_tmp = convQK(src_key / 1000.0, w['convK_w1'], w['convK_w2'])     # [b,512,16]
    query_tmp = convQK(src_query / 1000.0, w['convQ_w1'], w['convQ_w2'])  # [b,512,1]
    score = jnp.einsum('bdq,bdk->bqk', query_tmp, key_tmp) + 1e-15        # [b,1,16]
    dct_att = jnp.matmul(score, src_value)[:, 0].reshape(bs, -1, dct_n)   # [b,48,10]
    out0 = jnp.einsum('tk,bfk->btf', idct_m[:, :dct_n], dct_att)          # [b,35,48]

    gidx = np.array(list(range(INPUT_N - KS, INPUT_N)) + [INPUT_N - 1] * OUTPUT_N)
    input_gcn = src[:, gidx]                                              # [b,35,48]
    dct_initial = jnp.einsum('kt,btf->bfk', dct_m[:dct_n], input_gcn)     # [b,48,10]
    dct_in = jnp.concatenate([dct_initial, dct_att], axis=-1)             # [b,48,20]

    s1 = _gcn_fwd(w['gcn_s1'], dct_in)
    t1 = _gcn_fwd(w['gcn_t1'], jnp.swapaxes(s1, 1, 2))
    out1 = jnp.einsum('tk,bkf->btf', idct_m[:, :dct_n], t1[:, :dct_n, :])
    dct_out1 = jnp.einsum('kt,btf->bfk', dct_m[:dct_n], out1)
    dct_in2 = jnp.concatenate([dct_initial, dct_out1], axis=-1)
    s2 = _gcn_fwd(w['gcn_s2'], dct_in2)
    t2 = _gcn_fwd(w['gcn_t2'], jnp.swapaxes(s2, 1, 2))
    out2 = jnp.einsum('tk,bkf->btf', idct_m[:, :dct_n], t2[:, :dct_n, :])
    return out0, out1, out2, score


_PMAPPED = None


def _to_jnp_tree(x):
    if isinstance(x, dict):
        return {k: _to_jnp_tree(v) for k, v in x.items()}
    if isinstance(x, (list, tuple)):
        return [_to_jnp_tree(v) for v in x]
    return jnp.asarray(np.asarray(x, dtype=np.float32))


def _get_pmapped():
    global _PMAPPED
    if _PMAPPED is None:
        devs = jax.devices()[:N_CORES]
        _PMAPPED = jax.pmap(_forward, in_axes=(0, None), devices=devs)
    return _PMAPPED


def kernel(src, output_n, input_n, itera, convQ_w1, convQ_w2, convK_w1, convK_w2,
           gcn_s1, gcn_t1, gcn_s2, gcn_t2):
    src = np.asarray(src, dtype=np.float32)
    assert src.shape == (BATCH, INPUT_N, IN_F), src.shape

    weights = _to_jnp_tree({
        'convQ_w1': convQ_w1, 'convQ_w2': convQ_w2,
        'convK_w1': convK_w1, 'convK_w2': convK_w2,
        'gcn_s1': gcn_s1, 'gcn_t1': gcn_t1,
        'gcn_s2': gcn_s2, 'gcn_t2': gcn_t2,
    })

    # Shard batch across the 8 cores.
    b_local = BATCH // N_CORES
    src_sh = src.reshape(N_CORES, b_local, INPUT_N, IN_F)

    pf = _get_pmapped()
    out0, out1, out2, score = pf(src_sh, weights)

    # Gather/unshard: [8, 64, ...] -> [512, ...]; stack itera (=1) on axis 2.
    o0 = np.asarray(out0).reshape(BATCH, VL, IN_F)[:, :, None, :]
    o1 = np.asarray(out1).reshape(BATCH, VL, IN_F)[:, :, None, :]
    o2 = np.asarray(out2).reshape(BATCH, VL, IN_F)[:, :, None, :]
    sc = np.asarray(score).reshape(BATCH, 1, VN)
    return o0, o1, o2, sc


# revision 3
# speedup vs baseline: 2.0857x; 2.0857x over previous
"""Trainium kernel for nn_AttModel: DCT-window attention + spatio-temporal GCNs.

Strategy: pure data parallel (sharding_hint). Batch 512 is split across the
8 NeuronCores (64 samples each); all conv/GCN/attention weights are
replicated. The per-shard forward runs on each core via PJRT; results are
gathered and concatenated to the full batch.

Self-contained: all shapes/constants hardcoded (B=512, input_n=50,
output_n=25, itera=1, KS=10, DCT_N=10, D=512, F=48).
"""

import numpy as np
import jax
import jax.numpy as jnp

KS = 10
DCT_N = 10
D_MODEL = 512
IN_F = 48
BN_EPS = 1e-5
N_CORES = 8
BATCH = 512
INPUT_N = 50
OUTPUT_N = 25
VL = KS + OUTPUT_N          # 35
VN = INPUT_N - KS - OUTPUT_N + 1  # 16


def _get_dct_matrix(N):
    i = np.arange(N)
    k = np.arange(N)[:, None]
    w = np.full((N, 1), np.sqrt(2.0 / N))
    w[0] = np.sqrt(1.0 / N)
    dct = w * np.cos(np.pi * (i + 0.5) * k / N)
    return dct.astype(np.float32), np.linalg.inv(dct).astype(np.float32)


_DCT_NP, _IDCT_NP = _get_dct_matrix(VL)


def _conv1d(x, w):
    # x: [B, Cin, L], w: [Cout, Cin, K], VALID, stride 1, no bias
    return jax.lax.conv_general_dilated(
        x, w, (1,), 'VALID', dimension_numbers=('NCH', 'OIH', 'NCH'))


def _gconv(g, x):
    # y = att @ (x @ W) + b ; x: [B, node_n, F_in]
    return jnp.einsum('nm,bmf->bnf', g['att'], x @ g['w']) + g['b']


def _bn(y, gamma, beta):
    b, n, f = y.shape
    inv = 1.0 / jnp.sqrt(1.0 + BN_EPS)
    return ((y.reshape(b, -1) * inv) * gamma + beta).reshape(b, n, f)


def _gcn_fwd(p, x):
    y = jnp.tanh(_bn(_gconv(p['gc1'], x), p['bn1_g'], p['bn1_b']))
    for blk in p['blocks']:
        r = y
        z = jnp.tanh(_bn(_gconv(blk['gc1'], y), blk['bn1_g'], blk['bn1_b']))
        z = jnp.tanh(_bn(_gconv(blk['gc2'], z), blk['bn2_g'], blk['bn2_b']))
        y = z + r
    return _gconv(p['gc7'], y) + x


def _forward(src, w):
    """Per-shard forward. src: [b, 50, 48] fp32. Returns (out0, out1, out2, score)."""
    dct_m = jnp.asarray(_DCT_NP)
    idct_m = jnp.asarray(_IDCT_NP)
    dct_n = DCT_N
    bs = src.shape[0]

    src_key = jnp.swapaxes(src, 1, 2)[:, :, :INPUT_N - OUTPUT_N]   # [b,48,25]
    src_query = jnp.swapaxes(src, 1, 2)[:, :, -KS:]                # [b,48,10]
    widx = np.arange(VL)[None, :] + np.arange(VN)[:, None]         # [16,35]
    val = src[:, widx]                                             # [b,16,35,48]
    src_value = jnp.einsum('kt,bvtf->bvfk', dct_m[:dct_n], val).reshape(bs, VN, -1)

    def convQK(x, w1, w2):
        return jax.nn.relu(_conv1d(jax.nn.relu(_conv1d(x, w1)), w2))

    key_tmp = convQK(src_key / 1000.0, w['convK_w1'], w['convK_w2'])     # [b,512,16]
    query_tmp = convQK(src_query / 1000.0, w['convQ_w1'], w['convQ_w2'])  # [b,512,1]
    score = jnp.einsum('bdq,bdk->bqk', query_tmp, key_tmp) + 1e-15        # [b,1,16]
    dct_att = jnp.matmul(score, src_value)[:, 0].reshape(bs, -1, dct_n)   # [b,48,10]
    out0 = jnp.einsum('tk,bfk->btf', idct_m[:, :dct_n], dct_att)          # [b,35,48]

    gidx = np.array(list(range(INPUT_N - KS, INPUT_N)) + [INPUT_N - 1] * OUTPUT_N)
    input_gcn = src[:, gidx]                                              # [b,35,48]
    dct_initial = jnp.einsum('kt,btf->bfk', dct_m[:dct_n], input_gcn)     # [b,48,10]
    dct_in = jnp.concatenate([dct_initial, dct_att], axis=-1)             # [b,48,20]

    s1 = _gcn_fwd(w['gcn_s1'], dct_in)
    t1 = _gcn_fwd(w['gcn_t1'], jnp.swapaxes(s1, 1, 2))
    out1 = jnp.einsum('tk,bkf->btf', idct_m[:, :dct_n], t1[:, :dct_n, :])
    dct_out1 = jnp.einsum('kt,btf->bfk', dct_m[:dct_n], out1)
    dct_in2 = jnp.concatenate([dct_initial, dct_out1], axis=-1)
    s2 = _gcn_fwd(w['gcn_s2'], dct_in2)
    t2 = _gcn_fwd(w['gcn_t2'], jnp.swapaxes(s2, 1, 2))
    out2 = jnp.einsum('tk,bkf->btf', idct_m[:, :dct_n], t2[:, :dct_n, :])
    return out0, out1, out2, score


_PMAPPED = None
_WCACHE = None  # (np_leaves, device_weights)


def _np_tree(x):
    if isinstance(x, dict):
        return {k: _np_tree(v) for k, v in x.items()}
    if isinstance(x, (list, tuple)):
        return [_np_tree(v) for v in x]
    return np.asarray(x, dtype=np.float32)


def _get_pmapped():
    global _PMAPPED
    if _PMAPPED is None:
        devs = jax.devices()[:N_CORES]
        _PMAPPED = jax.pmap(_forward, in_axes=(0, 0), devices=devs)
    return _PMAPPED


def _device_weights(wnp):
    """Replicate weights onto the 8 cores once; reuse while bytes match."""
    global _WCACHE
    leaves = jax.tree_util.tree_leaves(wnp)
    if _WCACHE is not None:
        old_leaves, dev_w = _WCACHE
        if len(old_leaves) == len(leaves) and all(
                a.shape == b.shape and np.array_equal(a, b)
                for a, b in zip(old_leaves, leaves)):
            return dev_w
    devs = jax.devices()[:N_CORES]
    dev_w = jax.device_put_replicated(wnp, devs)
    _WCACHE = (leaves, dev_w)
    return dev_w


def kernel(src, output_n, input_n, itera, convQ_w1, convQ_w2, convK_w1, convK_w2,
           gcn_s1, gcn_t1, gcn_s2, gcn_t2):
    src = np.asarray(src, dtype=np.float32)
    assert src.shape == (BATCH, INPUT_N, IN_F), src.shape

    weights = _device_weights(_np_tree({
        'convQ_w1': convQ_w1, 'convQ_w2': convQ_w2,
        'convK_w1': convK_w1, 'convK_w2': convK_w2,
        'gcn_s1': gcn_s1, 'gcn_t1': gcn_t1,
        'gcn_s2': gcn_s2, 'gcn_t2': gcn_t2,
    }))

    # Shard batch across the 8 cores.
    b_local = BATCH // N_CORES
    src_sh = src.reshape(N_CORES, b_local, INPUT_N, IN_F)

    pf = _get_pmapped()
    out0, out1, out2, score = pf(src_sh, weights)

    # Gather/unshard: [8, 64, ...] -> [512, ...]; stack itera (=1) on axis 2.
    o0 = np.asarray(out0).reshape(BATCH, VL, IN_F)[:, :, None, :]
    o1 = np.asarray(out1).reshape(BATCH, VL, IN_F)[:, :, None, :]
    o2 = np.asarray(out2).reshape(BATCH, VL, IN_F)[:, :, None, :]
    sc = np.asarray(score).reshape(BATCH, 1, VN)
    return o0, o1, o2, sc


# revision 5
# speedup vs baseline: 3.6765x; 1.7627x over previous
"""Trainium kernel for nn_AttModel: DCT-window attention + spatio-temporal GCNs.

Strategy: pure data parallel (sharding_hint). Batch 512 is split across the
8 NeuronCores (64 samples each); all conv/GCN/attention weights are
replicated. The per-shard forward runs on each core via PJRT; results are
gathered and concatenated to the full batch.

Self-contained: all shapes/constants hardcoded (B=512, input_n=50,
output_n=25, itera=1, KS=10, DCT_N=10, D=512, F=48).
"""

import numpy as np
import jax
import jax.numpy as jnp

KS = 10
DCT_N = 10
D_MODEL = 512
IN_F = 48
BN_EPS = 1e-5
N_CORES = 8
BATCH = 512
INPUT_N = 50
OUTPUT_N = 25
VL = KS + OUTPUT_N          # 35
VN = INPUT_N - KS - OUTPUT_N + 1  # 16


def _get_dct_matrix(N):
    i = np.arange(N)
    k = np.arange(N)[:, None]
    w = np.full((N, 1), np.sqrt(2.0 / N))
    w[0] = np.sqrt(1.0 / N)
    dct = w * np.cos(np.pi * (i + 0.5) * k / N)
    return dct.astype(np.float32), np.linalg.inv(dct).astype(np.float32)


_DCT_NP, _IDCT_NP = _get_dct_matrix(VL)


def _conv1d(x, w):
    # x: [B, Cin, L], w: [Cout, Cin, K], VALID, stride 1, no bias
    return jax.lax.conv_general_dilated(
        x, w, (1,), 'VALID', dimension_numbers=('NCH', 'OIH', 'NCH'))


def _gconv(g, x):
    # y = att @ (x @ W) + b ; x: [B, node_n, F_in]
    return jnp.einsum('nm,bmf->bnf', g['att'], x @ g['w']) + g['b']


def _bn(y, gamma, beta):
    b, n, f = y.shape
    inv = 1.0 / jnp.sqrt(1.0 + BN_EPS)
    return ((y.reshape(b, -1) * inv) * gamma + beta).reshape(b, n, f)


def _gcn_fwd(p, x):
    y = jnp.tanh(_bn(_gconv(p['gc1'], x), p['bn1_g'], p['bn1_b']))
    for blk in p['blocks']:
        r = y
        z = jnp.tanh(_bn(_gconv(blk['gc1'], y), blk['bn1_g'], blk['bn1_b']))
        z = jnp.tanh(_bn(_gconv(blk['gc2'], z), blk['bn2_g'], blk['bn2_b']))
        y = z + r
    return _gconv(p['gc7'], y) + x


def _forward(src, w):
    """Per-shard forward. src: [b, 50, 48] fp32. Returns (out0, out1, out2, score)."""
    dct_m = jnp.asarray(_DCT_NP)
    idct_m = jnp.asarray(_IDCT_NP)
    dct_n = DCT_N
    bs = src.shape[0]

    src_key = jnp.swapaxes(src, 1, 2)[:, :, :INPUT_N - OUTPUT_N]   # [b,48,25]
    src_query = jnp.swapaxes(src, 1, 2)[:, :, -KS:]                # [b,48,10]
    widx = np.arange(VL)[None, :] + np.arange(VN)[:, None]         # [16,35]
    val = src[:, widx]                                             # [b,16,35,48]
    src_value = jnp.einsum('kt,bvtf->bvfk', dct_m[:dct_n], val).reshape(bs, VN, -1)

    def convQK(x, w1, w2):
        return jax.nn.relu(_conv1d(jax.nn.relu(_conv1d(x, w1)), w2))

    key_tmp = convQK(src_key / 1000.0, w['convK_w1'], w['convK_w2'])     # [b,512,16]
    query_tmp = convQK(src_query / 1000.0, w['convQ_w1'], w['convQ_w2'])  # [b,512,1]
    score = jnp.einsum('bdq,bdk->bqk', query_tmp, key_tmp) + 1e-15        # [b,1,16]
    dct_att = jnp.matmul(score, src_value)[:, 0].reshape(bs, -1, dct_n)   # [b,48,10]
    out0 = jnp.einsum('tk,bfk->btf', idct_m[:, :dct_n], dct_att)          # [b,35,48]

    gidx = np.array(list(range(INPUT_N - KS, INPUT_N)) + [INPUT_N - 1] * OUTPUT_N)
    input_gcn = src[:, gidx]                                              # [b,35,48]
    dct_initial = jnp.einsum('kt,btf->bfk', dct_m[:dct_n], input_gcn)     # [b,48,10]
    dct_in = jnp.concatenate([dct_initial, dct_att], axis=-1)             # [b,48,20]

    s1 = _gcn_fwd(w['gcn_s1'], dct_in)
    t1 = _gcn_fwd(w['gcn_t1'], jnp.swapaxes(s1, 1, 2))
    out1 = jnp.einsum('tk,bkf->btf', idct_m[:, :dct_n], t1[:, :dct_n, :])
    dct_out1 = jnp.einsum('kt,btf->bfk', dct_m[:dct_n], out1)
    dct_in2 = jnp.concatenate([dct_initial, dct_out1], axis=-1)
    s2 = _gcn_fwd(w['gcn_s2'], dct_in2)
    t2 = _gcn_fwd(w['gcn_t2'], jnp.swapaxes(s2, 1, 2))
    out2 = jnp.einsum('tk,bkf->btf', idct_m[:, :dct_n], t2[:, :dct_n, :])
    # Pack everything into one buffer per core: fewer host<->device round trips.
    return jnp.concatenate(
        [out0.reshape(bs, -1), out1.reshape(bs, -1), out2.reshape(bs, -1),
         score.reshape(bs, -1)], axis=1)  # [b, 3*35*48 + 16]


_PMAPPED = None
_WCACHE = None  # (np_leaves, device_weights)


def _np_tree(x):
    if isinstance(x, dict):
        return {k: _np_tree(v) for k, v in x.items()}
    if isinstance(x, (list, tuple)):
        return [_np_tree(v) for v in x]
    return np.asarray(x, dtype=np.float32)


def _get_pmapped():
    global _PMAPPED
    if _PMAPPED is None:
        devs = jax.devices()[:N_CORES]
        _PMAPPED = jax.pmap(_forward, in_axes=(0, 0), devices=devs)
    return _PMAPPED


def _device_weights(wnp):
    """Replicate weights onto the 8 cores once; reuse while bytes match."""
    global _WCACHE
    leaves = jax.tree_util.tree_leaves(wnp)
    if _WCACHE is not None:
        old_leaves, dev_w = _WCACHE
        if len(old_leaves) == len(leaves) and all(
                a.shape == b.shape and np.array_equal(a, b)
                for a, b in zip(old_leaves, leaves)):
            return dev_w
    devs = jax.devices()[:N_CORES]
    dev_w = jax.device_put_replicated(wnp, devs)
    _WCACHE = (leaves, dev_w)
    return dev_w


def kernel(src, output_n, input_n, itera, convQ_w1, convQ_w2, convK_w1, convK_w2,
           gcn_s1, gcn_t1, gcn_s2, gcn_t2):
    src = np.asarray(src, dtype=np.float32)
    assert src.shape == (BATCH, INPUT_N, IN_F), src.shape

    weights = _device_weights(_np_tree({
        'convQ_w1': convQ_w1, 'convQ_w2': convQ_w2,
        'convK_w1': convK_w1, 'convK_w2': convK_w2,
        'gcn_s1': gcn_s1, 'gcn_t1': gcn_t1,
        'gcn_s2': gcn_s2, 'gcn_t2': gcn_t2,
    }))

    # Shard batch across the 8 cores.
    b_local = BATCH // N_CORES
    src_sh = src.reshape(N_CORES, b_local, INPUT_N, IN_F)

    pf = _get_pmapped()
    packed = pf(src_sh, weights)  # [8, 64, 5056]

    # Gather/unshard: fetch the 8 shards in parallel, then unpack.
    import concurrent.futures as cf
    shards = [packed[i] for i in range(N_CORES)]
    with cf.ThreadPoolExecutor(max_workers=N_CORES) as ex:
        host = list(ex.map(np.asarray, shards))
    flat = np.concatenate(host, axis=0)  # [512, 5056]

    sz = VL * IN_F
    o0 = flat[:, :sz].reshape(BATCH, VL, IN_F)[:, :, None, :]
    o1 = flat[:, sz:2 * sz].reshape(BATCH, VL, IN_F)[:, :, None, :]
    o2 = flat[:, 2 * sz:3 * sz].reshape(BATCH, VL, IN_F)[:, :, None, :]
    sc = np.ascontiguousarray(flat[:, 3 * sz:]).reshape(BATCH, 1, VN)
    return o0, o1, o2, sc
